# revision 25
# baseline (speedup 1.0000x reference)
"""Trainium2 Bass kernel for the moe_routing classifier problem (v2).

Computation (per batch row b, class c):
  cos[b,c,s]  = cosine(emb[b], weight[c,s])            (64 sub-prototypes)
  top-8 over s, softmax weights w, protos = sum_k w_k * weight[c, idx_k]
  out[b,c]    = ((1 + cosine(protos, emb[b])) / 2 + 1e-8) / 0.1

Key algebra (per (b,c); E = masked exp of scores, any per-(b,c) scale of E
cancels between numerator and denominator):
  u          = 0.5/||emb_b|| * dotn + BIG   (dotn = emb . What, normalized W)
  x          = u * 1[u >= thr8]             (thr8 = 8th largest u per class)
  Et         = exp(x^T - BIG)               (unselected -> e^-BIG ~ 1e-13)
  h*d2z      = sum_s ||w_s|| * Et * (x^T - BIG)
  np2z       = Et^T G Et  via pair-block-diag Gram matmuls
  out        = 10 * (h*d2z) / sqrt(np2z) + 5 + 1e-7   (all ||emb|| cancel)

Both reductions over s run on the PE array: one accumulating matmul per
class-pair against a [128, 64] selector whose cols 0:32 hold ||w_s||*onehot
(for d2z) and cols 32:64 hold onehot (for np2z).

Sharding: classes split across 8 cores (32 each), emb replicated; each core
writes a [1024, 32] slice of the output.
"""

import numpy as np

B, D, C, S = 1024, 128, 256, 64
NCORES = 8
C_LOC = C // NCORES        # 32 classes per core
CS = C_LOC * S             # 2048 anchor rows per core
P = 128                    # partitions
NBT = B // P               # 8 batch tiles
NWT = CS // P              # 16 weight tiles (= class pairs)
NPAIR = NWT
BIG = 30.0                 # separation constant for masked exp
OUT_BIAS = 5.0 + 1e-7      # ((1+x)/2 + 1e-8) / 0.1 = 5x + 5 + 1e-7

_CACHE = {}


def build_nc():
    import concourse.bass as bass
    import concourse.tile as tile
    from concourse import bacc, mybir
    from concourse.masks import make_identity
    from contextlib import ExitStack

    f32 = mybir.dt.float32
    f32r = mybir.dt.float32r
    bf16 = mybir.dt.bfloat16
    AF = mybir.ActivationFunctionType
    ALU = mybir.AluOpType

    nc = bacc.Bacc(None, target_bir_lowering=False)
    emb_d = nc.dram_tensor("emb", [B, D], f32, kind="ExternalInput")
    w_d = nc.dram_tensor("weight", [CS, D], f32, kind="ExternalInput")
    out_d = nc.dram_tensor("out", [B, C_LOC], f32, kind="ExternalOutput")

    with tile.TileContext(nc) as tc, ExitStack() as ctx:
        sing = ctx.enter_context(tc.tile_pool(name="sing", bufs=1))
        work = ctx.enter_context(tc.tile_pool(name="work", bufs=2))
        small = ctx.enter_context(tc.tile_pool(name="small", bufs=2))
        ps_a = ctx.enter_context(tc.tile_pool(name="ps_a", bufs=1, space="PSUM"))
        ps_tr = ctx.enter_context(tc.tile_pool(name="ps_tr", bufs=2, space="PSUM"))
        ps_h = ctx.enter_context(tc.tile_pool(name="ps_h", bufs=1, space="PSUM"))
        ps_sel = ctx.enter_context(tc.tile_pool(name="ps_sel", bufs=1, space="PSUM"))

        ident = sing.tile([P, P], f32)
        make_identity(nc, ident[:])
        negbig = sing.tile([P, 1], f32)
        nc.vector.memset(negbig[:], -BIG)
        bigt = sing.tile([P, 1], f32)
        nc.vector.memset(bigt[:], BIG)

        # ---------------- load inputs (emb first, separate DMA queues) ----
        En = sing.tile([P, NBT, D], f32)
        nc.sync.dma_start(En[:], emb_d[:].rearrange("(t p) d -> p t d", p=P))
        Wn = sing.tile([P, NWT, D], f32)
        nc.gpsimd.dma_start(Wn[:], w_d[:].rearrange("(t p) d -> p t d", p=P))

        # ---------------- emb-side preproc ----------------
        # squares on DVE (mult+reduce) to keep the scalar engine free
        esqt = sing.tile([P, B], f32)
        nc.vector.tensor_mul(esqt[:], En[:].rearrange("p t d -> p (t d)"),
                             En[:].rearrange("p t d -> p (t d)"))
        esq = sing.tile([P, NBT], f32)
        nc.vector.tensor_reduce(esq[:], esqt[:].rearrange("p (t d) -> p t d", d=D),
                                axis=mybir.AxisListType.X, op=ALU.add)
        ne = sing.tile([P, NBT], f32)          # ||emb||  (sqrt table set)
        nc.scalar.activation(ne[:], esq[:], AF.Sqrt)
        ine = sing.tile([P, NBT], f32)
        hine = sing.tile([P, NBT], f32)        # 0.5/||emb||
        nc.vector.reciprocal_approx_accurate(ine[:], ne[:], hine[:])
        nc.vector.tensor_scalar_mul(hine[:], ine[:], 0.5)

        embT = sing.tile([P, B], f32r)         # emb^T [d, b]
        for g in range(2):
            pst = ps_tr.tile([P, 4 * P], f32, tag="tr")
            for t in range(4):
                nc.tensor.transpose(pst[:, t * P : (t + 1) * P],
                                    En[:, 4 * g + t], ident[:])
            nc.scalar.copy(embT[:, g * 512 : (g + 1) * 512], pst[:])

        # ---------------- weight-side preproc ----------------
        wsqt = sing.tile([P, CS], f32)
        nc.vector.tensor_mul(wsqt[:], Wn[:].rearrange("p t d -> p (t d)"),
                             Wn[:].rearrange("p t d -> p (t d)"))
        nwsq = sing.tile([P, NWT], f32)        # ||w_row||^2 row-tiled
        nc.vector.tensor_reduce(nwsq[:], wsqt[:].rearrange("p (t d) -> p t d", d=D),
                                axis=mybir.AxisListType.X, op=ALU.add)
        nw_row = sing.tile([P, NWT], f32)      # ||w_row||
        nc.scalar.activation(nw_row[:], nwsq[:], AF.Sqrt)
        inw_row = sing.tile([P, NWT], f32)
        inw_scr = sing.tile([P, NWT], f32)
        nc.vector.reciprocal_approx_accurate(inw_row[:], nw_row[:], inw_scr[:])

        # normalized anchors (one wide mul with broadcast scale), transposed
        Vn = sing.tile([P, NWT, D], f32)
        VT = sing.tile([P, CS], f32r)
        inw_b = inw_row[:, :, None].to_broadcast([P, NWT, D])
        nc.vector.tensor_mul(Vn[:], Wn[:], inw_b)
        for g in range(4):
            pst = ps_tr.tile([P, 4 * P], f32, tag="tr")
            for t in range(4):
                nc.tensor.transpose(pst[:, t * P : (t + 1) * P],
                                    Vn[:, 4 * g + t], ident[:])
            nc.vector.tensor_copy(VT[:, g * 512 : (g + 1) * 512], pst[:])

        # selector matrix per pair: cols 0:32 = ||w||*onehot, 32:64 = onehot
        # nonzero col for (pair q, parity) is 2q+par -> flat idx 66q + par
        selb = sing.tile([P, NPAIR, 2 * C_LOC], bf16)
        nc.vector.memset(selb[:], 0.0)
        self_flat = selb[:].rearrange("p q c -> p (q c)")
        for par in range(2):
            psl = slice(par * 64, par * 64 + 64)
            a0 = self_flat[psl, par : par + 1]
            dst_nw = bass.AP(tensor=a0.tensor, offset=a0.offset,
                             ap=[a0.ap[0], [2 * C_LOC + 2, NPAIR]])
            nc.vector.tensor_copy(dst_nw, nw_row[psl, :])
            a1 = self_flat[psl, C_LOC + par : C_LOC + par + 1]
            dst_one = bass.AP(tensor=a1.tensor, offset=a1.offset,
                              ap=[a1.ap[0], [2 * C_LOC + 2, NPAIR]])
            nc.vector.memset(dst_one, 1.0)

        tiles = {}
        pb = {}

        def emit_iter(ba, bb, bc):
            """One pipeline iteration: stage-A chunks of tile ba interleaved
            with stage-B groups of tile bb, then stage-C of tile bc."""
            # --- allocations ---
            if ba is not None:
                u = work.tile([P, CS], f32, tag="u", bufs=3)
            if bb is not None:
                x0 = tiles.pop(bb)
                Et = work.tile([P, NPAIR, P], bf16, tag="Et", bufs=2)
                prods = work.tile([P, NPAIR, 2 * P], bf16, tag="prods", bufs=2)
                Et3 = Et[:]
                pr3 = prods[:]
                hps = ps_h.tile([P, CS], f32, tag="h")
                pb[bb] = prods
            # --- interleaved A-chunks and B transpose groups (4 each) ---
            for g in range(4):
                if ba is not None:
                    js = slice(g * 512, (g + 1) * 512)
                    dps = ps_a.tile([P, 512], f32, tag="mm")
                    nc.tensor.matmul(dps[:], embT[:, ba * P : (ba + 1) * P],
                                     VT[:, js])
                    nc.scalar.activation(u[:, js], dps[:], AF.Copy,
                                         bias=BIG, scale=hine[:, ba : ba + 1])
                if bb is not None:
                    qs = slice(4 * g, 4 * g + 4)
                    xps = ps_tr.tile([P, 512], f32, tag="tr")
                    for j in range(4):
                        q = 4 * g + j
                        nc.tensor.transpose(xps[:, j * P : (j + 1) * P],
                                            x0[:, q * P : (q + 1) * P],
                                            ident[:])
                    nc.scalar.activation(
                        Et3[:, qs, :].rearrange("p q x -> p (q x)"),
                        xps[:], AF.Exp, bias=negbig[:])
                    x3 = xps[:].rearrange("p (q x) -> p q x", x=P)
                    nc.vector.scalar_tensor_tensor(
                        pr3[:, qs, 0:P], x3, BIG, Et3[:, qs, :],
                        op0=ALU.subtract, op1=ALU.mult)
            # --- B: H matmuls (after Et groups exist) ---
            if bb is not None:
                for q in range(NPAIR):
                    nc.tensor.matmul(hps[:, q * P : (q + 1) * P],
                                     GPb[:, q, :], Et3[:, q, :])
            # --- A: selection ---
            if ba is not None:
                mx8 = small.tile([P, C_LOC * 8], f32, tag="mx8", bufs=2)
                for c in range(C_LOC):
                    nc.vector.max(out=mx8[:, c * 8 : (c + 1) * 8],
                                  in_=u[:, c * S : (c + 1) * S])
                mask = work.tile([P, CS], f32, tag="mask", bufs=2)
                u3 = u[:].rearrange("p (c s) -> p c s", s=S)
                m3 = mask[:].rearrange("p (c s) -> p c s", s=S)
                thr = mx8[:].rearrange("p (c k) -> p c k", k=8)[:, :, 7]
                thr_b = thr[:, :, None].to_broadcast([P, C_LOC, S])
                nc.vector.tensor_tensor(m3, u3, thr_b, ALU.is_ge)
                x0a = work.tile([P, CS], f32, tag="x0", bufs=3)
                nc.gpsimd.tensor_mul(x0a[:], u[:], mask[:])
                tiles[ba] = x0a
            # --- B: prod_n ---
            if bb is not None:
                h3 = hps[:].rearrange("p (q x) -> p q x", x=P)
                nc.vector.tensor_mul(pr3[:, :, P : 2 * P], Et3, h3)
            # --- C: selector reduction + evac ---
            if bc is not None:
                prc = pb.pop(bc)[:]
                selps = ps_sel.tile([64, 2 * P], f32, tag="sel")
                for q in range(NPAIR):
                    nc.tensor.matmul(selps[:], selb[:, q, :], prc[:, q, :],
                                     start=(q == 0), stop=(q == NPAIR - 1))
                nc.scalar.copy(dznp[:, bc, :], selps[:])

        # per-class raw Gram matrices, packed block-diagonal per pair
        def build_gram():
            WTb = sing.tile([P, CS], bf16)
            for g in range(4):
                pst = ps_tr.tile([P, 512], f32, tag="tr")
                for t in range(4):
                    nc.tensor.transpose(pst[:, t * P : (t + 1) * P],
                                        Wn[:, 4 * g + t], ident[:])
                nc.scalar.copy(WTb[:, g * 512 : (g + 1) * 512], pst[:])
            Gt = sing.tile([S, CS], bf16)
            for g in range(4):
                psg = ps_h.tile([P, 512], f32, tag="h")
                for i in range(8):
                    c = 8 * g + i
                    cs = slice(c * S, (c + 1) * S)
                    nc.tensor.matmul(psg[0:S, i * S : (i + 1) * S],
                                     WTb[:, cs], WTb[:, cs])
                nc.scalar.copy(Gt[:, g * 512 : (g + 1) * 512], psg[0:S, :])
            GP = sing.tile([P, NPAIR, P], bf16)
            nc.vector.memset(GP[:], 0.0)
            gt3 = Gt[:].rearrange("p (q j) -> p q j", j=2 * S)
            gp3 = GP[:]
            nc.sync.dma_start(gp3[0:S, :, 0:S], gt3[:, :, 0:S])
            nc.sync.dma_start(gp3[S : 2 * S, :, S : 2 * S], gt3[:, :, S : 2 * S])
            return GP

        dznp = sing.tile([64, NBT, 2 * P], f32)

        # warm the Exp table before the steady loop (sqrt uses are done)
        texp = sing.tile([P, 1], f32)
        nc.scalar.activation(texp[:], bigt[:], AF.Exp)

        # ---------------- software-pipelined main loop ----------------
        emit_iter(0, None, None)
        emit_iter(1, None, None)
        GPb = build_gram()
        for k in range(2, NBT):
            emit_iter(k, k - 2, k - 3 if k >= 3 else None)
        emit_iter(None, NBT - 2, NBT - 3)
        emit_iter(None, NBT - 1, NBT - 2)
        emit_iter(None, None, NBT - 1)

        # ---------------- tail ----------------
        # out = 10 * (h*d2z) / sqrt(np2z) + OUT_BIAS
        d2zv = dznp[0:C_LOC, :, 0:P]                       # [32, 8, 128]
        np2v = dznp[C_LOC : 2 * C_LOC, :, P : 2 * P]       # [32, 8, 128]
        nps = sing.tile([C_LOC, NBT, P], f32)
        nc.scalar.activation(nps[:], np2v, AF.Sqrt, scale=0.01)
        rs = sing.tile([C_LOC, NBT, P], f32)
        t1 = sing.tile([C_LOC, NBT, P], f32)
        nc.vector.reciprocal_approx_accurate(rs[:], nps[:], t1[:])
        nc.vector.tensor_mul(t1[:], d2zv, rs[:])
        osb = sing.tile([P, NBT, C_LOC], f32)
        for g in range(2):
            pst = ps_tr.tile([P, 512], f32, tag="tr")
            for t in range(4):
                bt = 4 * g + t
                nc.tensor.transpose(pst[:, t * C_LOC : (t + 1) * C_LOC],
                                    t1[:, bt, :], ident[0:C_LOC, 0:C_LOC])
            nc.scalar.activation(
                osb[:, 4 * g : 4 * g + 4, :].rearrange("p t c -> p (t c)"),
                pst[:, 0 : 4 * C_LOC], AF.Copy, bias=OUT_BIAS)
        nc.sync.dma_start(out_d[:].rearrange("(t p) c -> p t c", p=P), osb[:])

    nc.compile()
    return nc


def _get_nc():
    if "nc" not in _CACHE:
        _CACHE["nc"] = build_nc()
    return _CACHE["nc"]


def kernel(emb: np.ndarray, weight: np.ndarray) -> np.ndarray:
    from concourse.bass_utils import run_bass_kernel_spmd

    emb = np.ascontiguousarray(np.asarray(emb, dtype=np.float32))
    weight = np.ascontiguousarray(np.asarray(weight, dtype=np.float32))
    assert emb.shape == (B, D) and weight.shape == (C, S, D)

    nc = _get_nc()
    in_maps = [
        {
            "emb": emb,
            "weight": np.ascontiguousarray(
                weight[i * C_LOC : (i + 1) * C_LOC].reshape(CS, D)
            ),
        }
        for i in range(NCORES)
    ]
    res = run_bass_kernel_spmd(nc, in_maps, core_ids=list(range(NCORES)))
    return np.concatenate(
        [res.results[i]["out"] for i in range(NCORES)], axis=1
    )


# revision 28
# speedup vs baseline: 1.0524x; 1.0524x over previous
"""Trainium2 Bass kernel for the moe_routing classifier problem (v2).

Computation (per batch row b, class c):
  cos[b,c,s]  = cosine(emb[b], weight[c,s])            (64 sub-prototypes)
  top-8 over s, softmax weights w, protos = sum_k w_k * weight[c, idx_k]
  out[b,c]    = ((1 + cosine(protos, emb[b])) / 2 + 1e-8) / 0.1

Key algebra (per (b,c); E = masked exp of scores, any per-(b,c) scale of E
cancels between numerator and denominator):
  u          = 0.5/||emb_b|| * dotn + BIG   (dotn = emb . What, normalized W)
  x          = u * 1[u >= thr8]             (thr8 = 8th largest u per class)
  Et         = exp(x^T - BIG)               (unselected -> e^-BIG ~ 1e-13)
  h*d2z      = sum_s ||w_s|| * Et * (x^T - BIG)
  np2z       = Et^T G Et  via pair-block-diag Gram matmuls
  out        = 10 * (h*d2z) / sqrt(np2z) + 5 + 1e-7   (all ||emb|| cancel)

Both reductions over s run on the PE array: one accumulating matmul per
class-pair against a [128, 64] selector whose cols 0:32 hold ||w_s||*onehot
(for d2z) and cols 32:64 hold onehot (for np2z).

Sharding: classes split across 8 cores (32 each), emb replicated; each core
writes a [1024, 32] slice of the output.
"""

import numpy as np

B, D, C, S = 1024, 128, 256, 64
NCORES = 8
C_LOC = C // NCORES        # 32 classes per core
CS = C_LOC * S             # 2048 anchor rows per core
P = 128                    # partitions
NBT = B // P               # 8 batch tiles
NWT = CS // P              # 16 weight tiles (= class pairs)
NPAIR = NWT
BIG = 30.0                 # separation constant for masked exp
OUT_BIAS = 5.0 + 1e-7      # ((1+x)/2 + 1e-8) / 0.1 = 5x + 5 + 1e-7

_CACHE = {}


def build_nc():
    import concourse.bass as bass
    import concourse.tile as tile
    from concourse import bacc, mybir
    from concourse.masks import make_identity
    from contextlib import ExitStack

    f32 = mybir.dt.float32
    f32r = mybir.dt.float32r
    bf16 = mybir.dt.bfloat16
    AF = mybir.ActivationFunctionType
    ALU = mybir.AluOpType

    nc = bacc.Bacc(None, target_bir_lowering=False)
    emb_d = nc.dram_tensor("emb", [B, D], f32, kind="ExternalInput")
    w_d = nc.dram_tensor("weight", [CS, D], f32, kind="ExternalInput")
    out_d = nc.dram_tensor("out", [B, C_LOC], f32, kind="ExternalOutput")

    with tile.TileContext(nc) as tc, ExitStack() as ctx:
        sing = ctx.enter_context(tc.tile_pool(name="sing", bufs=1))
        work = ctx.enter_context(tc.tile_pool(name="work", bufs=2))
        small = ctx.enter_context(tc.tile_pool(name="small", bufs=2))
        ps_a = ctx.enter_context(tc.tile_pool(name="ps_a", bufs=1, space="PSUM"))
        ps_tr = ctx.enter_context(tc.tile_pool(name="ps_tr", bufs=2, space="PSUM"))
        ps_h = ctx.enter_context(tc.tile_pool(name="ps_h", bufs=1, space="PSUM"))
        ps_sel = ctx.enter_context(tc.tile_pool(name="ps_sel", bufs=1, space="PSUM"))

        ident = sing.tile([P, P], f32)
        make_identity(nc, ident[:])
        negbig = sing.tile([P, 1], f32)
        nc.vector.memset(negbig[:], -BIG)
        bigt = sing.tile([P, 1], f32)
        nc.vector.memset(bigt[:], BIG)

        # ---------------- load inputs (emb first, separate DMA queues) ----
        En = sing.tile([P, NBT, D], f32)
        nc.sync.dma_start(En[:], emb_d[:].rearrange("(t p) d -> p t d", p=P))
        Wn = sing.tile([P, NWT, D], f32)
        nc.gpsimd.dma_start(Wn[:], w_d[:].rearrange("(t p) d -> p t d", p=P))

        # ---------------- emb-side preproc ----------------
        # squares on DVE (mult+reduce) to keep the scalar engine free
        esqt = sing.tile([P, B], f32)
        nc.vector.tensor_mul(esqt[:], En[:].rearrange("p t d -> p (t d)"),
                             En[:].rearrange("p t d -> p (t d)"))
        esq = sing.tile([P, NBT], f32)
        nc.vector.tensor_reduce(esq[:], esqt[:].rearrange("p (t d) -> p t d", d=D),
                                axis=mybir.AxisListType.X, op=ALU.add)
        ne = sing.tile([P, NBT], f32)          # ||emb||  (sqrt table set)
        nc.scalar.activation(ne[:], esq[:], AF.Sqrt)
        ine = sing.tile([P, NBT], f32)
        hine = sing.tile([P, NBT], f32)        # 0.5/||emb||
        nc.vector.reciprocal_approx_accurate(ine[:], ne[:], hine[:])
        nc.vector.tensor_scalar_mul(hine[:], ine[:], 0.5)

        embT = sing.tile([P, B], f32r)         # emb^T [d, b]
        for g in range(2):
            pst = ps_tr.tile([P, 4 * P], f32, tag="tr")
            for t in range(4):
                nc.tensor.transpose(pst[:, t * P : (t + 1) * P],
                                    En[:, 4 * g + t], ident[:])
            nc.scalar.copy(embT[:, g * 512 : (g + 1) * 512], pst[:])

        # ---------------- weight-side preproc ----------------
        wsqt = sing.tile([P, CS], f32)
        nc.vector.tensor_mul(wsqt[:], Wn[:].rearrange("p t d -> p (t d)"),
                             Wn[:].rearrange("p t d -> p (t d)"))
        nwsq = sing.tile([P, NWT], f32)        # ||w_row||^2 row-tiled
        nc.vector.tensor_reduce(nwsq[:], wsqt[:].rearrange("p (t d) -> p t d", d=D),
                                axis=mybir.AxisListType.X, op=ALU.add)
        nw_row = sing.tile([P, NWT], f32)      # ||w_row||
        nc.scalar.activation(nw_row[:], nwsq[:], AF.Sqrt)
        inw_row = sing.tile([P, NWT], f32)
        inw_scr = sing.tile([P, NWT], f32)
        nc.vector.reciprocal_approx_accurate(inw_row[:], nw_row[:], inw_scr[:])

        # normalized anchors (one wide mul with broadcast scale), transposed
        Vn = sing.tile([P, NWT, D], f32)
        VT = sing.tile([P, CS], f32r)
        inw_b = inw_row[:, :, None].to_broadcast([P, NWT, D])
        nc.vector.tensor_mul(Vn[:], Wn[:], inw_b)
        for g in range(4):
            pst = ps_tr.tile([P, 4 * P], f32, tag="tr")
            for t in range(4):
                nc.tensor.transpose(pst[:, t * P : (t + 1) * P],
                                    Vn[:, 4 * g + t], ident[:])
            nc.scalar.copy(VT[:, g * 512 : (g + 1) * 512], pst[:])

        # selector matrix per pair: cols 0:32 = ||w||*onehot, 32:64 = onehot
        # nonzero col for (pair q, parity) is 2q+par -> flat idx 66q + par
        selb = sing.tile([P, NPAIR, 2 * C_LOC], bf16)
        nc.vector.memset(selb[:], 0.0)
        self_flat = selb[:].rearrange("p q c -> p (q c)")
        for par in range(2):
            psl = slice(par * 64, par * 64 + 64)
            a0 = self_flat[psl, par : par + 1]
            dst_nw = bass.AP(tensor=a0.tensor, offset=a0.offset,
                             ap=[a0.ap[0], [2 * C_LOC + 2, NPAIR]])
            nc.vector.tensor_copy(dst_nw, nw_row[psl, :])
            a1 = self_flat[psl, C_LOC + par : C_LOC + par + 1]
            dst_one = bass.AP(tensor=a1.tensor, offset=a1.offset,
                              ap=[a1.ap[0], [2 * C_LOC + 2, NPAIR]])
            nc.vector.memset(dst_one, 1.0)

        tiles = {}
        pb = {}

        def emit_iter(ba, bb, bc):
            """One pipeline iteration: stage-A chunks of tile ba interleaved
            with stage-B groups of tile bb, then stage-C of tile bc."""
            # --- allocations ---
            if ba is not None:
                u = work.tile([P, CS], f32, tag="u", bufs=3)
            if bb is not None:
                x0 = tiles.pop(bb)
                Et = work.tile([P, NPAIR, P], bf16, tag="Et", bufs=2)
                xs = work.tile([P, NPAIR, P], bf16, tag="xs", bufs=2)
                prods = work.tile([P, NPAIR, 2 * P], bf16, tag="prods", bufs=2)
                Et3 = Et[:]
                xs3 = xs[:]
                pr3 = prods[:]
                hps = ps_h.tile([P, CS], f32, tag="h")
                pb[bb] = prods
            # --- interleaved A-chunks and B transpose groups (4 each) ---
            for g in range(4):
                if ba is not None:
                    js = slice(g * 512, (g + 1) * 512)
                    dps = ps_a.tile([P, 512], f32, tag="mm")
                    nc.tensor.matmul(dps[:], embT[:, ba * P : (ba + 1) * P],
                                     VT[:, js])
                    nc.scalar.activation(u[:, js], dps[:], AF.Copy,
                                         bias=BIG, scale=hine[:, ba : ba + 1])
                if bb is not None:
                    qs = slice(4 * g, 4 * g + 4)
                    xps = ps_tr.tile([P, 512], f32, tag="tr")
                    for j in range(4):
                        q = 4 * g + j
                        nc.tensor.transpose(xps[:, j * P : (j + 1) * P],
                                            x0[:, q * P : (q + 1) * P],
                                            ident[:])
                    nc.scalar.activation(
                        Et3[:, qs, :].rearrange("p q x -> p (q x)"),
                        xps[:], AF.Exp, bias=negbig[:])
                    nc.scalar.activation(
                        xs3[:, qs, :].rearrange("p q x -> p (q x)"),
                        xps[:], AF.Copy, bias=-BIG)
            # --- B: prod_d on gpsimd (all-SBUF), H matmuls on PE ---
            if bb is not None:
                nc.gpsimd.tensor_mul(pr3[:, :, 0:P], xs3, Et3)
                for q in range(NPAIR):
                    nc.tensor.matmul(hps[:, q * P : (q + 1) * P],
                                     GPb[:, q, :], Et3[:, q, :])
            # --- A: selection ---
            if ba is not None:
                mx8 = small.tile([P, C_LOC * 8], f32, tag="mx8", bufs=2)
                for c in range(C_LOC):
                    nc.vector.max(out=mx8[:, c * 8 : (c + 1) * 8],
                                  in_=u[:, c * S : (c + 1) * S])
                mask = work.tile([P, CS], f32, tag="mask", bufs=2)
                u3 = u[:].rearrange("p (c s) -> p c s", s=S)
                m3 = mask[:].rearrange("p (c s) -> p c s", s=S)
                thr = mx8[:].rearrange("p (c k) -> p c k", k=8)[:, :, 7]
                thr_b = thr[:, :, None].to_broadcast([P, C_LOC, S])
                nc.vector.tensor_tensor(m3, u3, thr_b, ALU.is_ge)
                x0a = work.tile([P, CS], f32, tag="x0", bufs=3)
                nc.gpsimd.tensor_mul(x0a[:], u[:], mask[:])
                tiles[ba] = x0a
            # --- B: prod_n ---
            if bb is not None:
                h3 = hps[:].rearrange("p (q x) -> p q x", x=P)
                nc.vector.tensor_mul(pr3[:, :, P : 2 * P], Et3, h3)
            # --- C: selector reduction + evac ---
            if bc is not None:
                prc = pb.pop(bc)[:]
                selps = ps_sel.tile([64, 2 * P], f32, tag="sel")
                for q in range(NPAIR):
                    nc.tensor.matmul(selps[:], selb[:, q, :], prc[:, q, :],
                                     start=(q == 0), stop=(q == NPAIR - 1))
                nc.scalar.copy(dznp[:, bc, :], selps[:])

        # per-class raw Gram matrices, packed block-diagonal per pair
        def build_gram():
            WTb = sing.tile([P, CS], bf16)
            for g in range(4):
                pst = ps_tr.tile([P, 512], f32, tag="tr")
                for t in range(4):
                    nc.tensor.transpose(pst[:, t * P : (t + 1) * P],
                                        Wn[:, 4 * g + t], ident[:])
                nc.scalar.copy(WTb[:, g * 512 : (g + 1) * 512], pst[:])
            Gt = sing.tile([S, CS], bf16)
            for g in range(4):
                psg = ps_h.tile([P, 512], f32, tag="h")
                for i in range(8):
                    c = 8 * g + i
                    cs = slice(c * S, (c + 1) * S)
                    nc.tensor.matmul(psg[0:S, i * S : (i + 1) * S],
                                     WTb[:, cs], WTb[:, cs])
                nc.scalar.copy(Gt[:, g * 512 : (g + 1) * 512], psg[0:S, :])
            GP = sing.tile([P, NPAIR, P], bf16)
            nc.vector.memset(GP[:], 0.0)
            gt3 = Gt[:].rearrange("p (q j) -> p q j", j=2 * S)
            gp3 = GP[:]
            nc.sync.dma_start(gp3[0:S, :, 0:S], gt3[:, :, 0:S])
            nc.sync.dma_start(gp3[S : 2 * S, :, S : 2 * S], gt3[:, :, S : 2 * S])
            return GP

        dznp = sing.tile([64, NBT, 2 * P], f32)

        # warm the Exp table before the steady loop (sqrt uses are done)
        texp = sing.tile([P, 1], f32)
        nc.scalar.activation(texp[:], bigt[:], AF.Exp)

        # ---------------- software-pipelined main loop ----------------
        emit_iter(0, None, None)
        emit_iter(1, None, None)
        GPb = build_gram()
        for k in range(2, NBT):
            emit_iter(k, k - 2, k - 3 if k >= 3 else None)
        emit_iter(None, NBT - 2, NBT - 3)
        emit_iter(None, NBT - 1, NBT - 2)
        emit_iter(None, None, NBT - 1)

        # ---------------- tail ----------------
        # out = 10 * (h*d2z) / sqrt(np2z) + OUT_BIAS
        d2zv = dznp[0:C_LOC, :, 0:P]                       # [32, 8, 128]
        np2v = dznp[C_LOC : 2 * C_LOC, :, P : 2 * P]       # [32, 8, 128]
        nps = sing.tile([C_LOC, NBT, P], f32)
        nc.scalar.activation(nps[:], np2v, AF.Sqrt, scale=0.01)
        rs = sing.tile([C_LOC, NBT, P], f32)
        t1 = sing.tile([C_LOC, NBT, P], f32)
        nc.vector.reciprocal_approx_accurate(rs[:], nps[:], t1[:])
        nc.vector.tensor_mul(t1[:], d2zv, rs[:])
        osb = sing.tile([P, NBT, C_LOC], f32)
        for g in range(2):
            pst = ps_tr.tile([P, 512], f32, tag="tr")
            for t in range(4):
                bt = 4 * g + t
                nc.tensor.transpose(pst[:, t * C_LOC : (t + 1) * C_LOC],
                                    t1[:, bt, :], ident[0:C_LOC, 0:C_LOC])
            nc.scalar.activation(
                osb[:, 4 * g : 4 * g + 4, :].rearrange("p t c -> p (t c)"),
                pst[:, 0 : 4 * C_LOC], AF.Copy, bias=OUT_BIAS)
        nc.sync.dma_start(out_d[:].rearrange("(t p) c -> p t c", p=P), osb[:])

    nc.compile()
    return nc


def _get_nc():
    if "nc" not in _CACHE:
        _CACHE["nc"] = build_nc()
    return _CACHE["nc"]


def kernel(emb: np.ndarray, weight: np.ndarray) -> np.ndarray:
    from concourse.bass_utils import run_bass_kernel_spmd

    emb = np.ascontiguousarray(np.asarray(emb, dtype=np.float32))
    weight = np.ascontiguousarray(np.asarray(weight, dtype=np.float32))
    assert emb.shape == (B, D) and weight.shape == (C, S, D)

    nc = _get_nc()
    in_maps = [
        {
            "emb": emb,
            "weight": np.ascontiguousarray(
                weight[i * C_LOC : (i + 1) * C_LOC].reshape(CS, D)
            ),
        }
        for i in range(NCORES)
    ]
    res = run_bass_kernel_spmd(nc, in_maps, core_ids=list(range(NCORES)))
    return np.concatenate(
        [res.results[i]["out"] for i in range(NCORES)], axis=1
    )
